# revision 2
# baseline (speedup 1.0000x reference)
"""Trainium2 Bass kernel for nn_CBF (dense MLP forward + Jacobian row).

Math: the reference computes HJH = [h, Jh] where
  h  = MLP(x_norm)            (scalar)
  Jh = Wout @ W3 @ D2 @ W2 @ D1 @ W1 @ D0 @ (W0 / x_range)   (1 x n row)
Everything is linear in the Jacobian chain, so Jh collapses to
  r0 @ W0 / x_range   with  r0 = left-chain of tiny 128-dim vectors.
Only two passes over the big W0 (128 x 131072) are needed:
  matvec1: V0 = x_norm @ W0.T    (contraction over n)
  matvec2: J  = r0 @ W0          (contraction over h)
W0 is sharded along n over 8 cores (16K cols, 8.4 MB per core per pass).

Two kernel launches (a cross-core AllReduce measured ~114 us on this stack,
vs ~40 us for a full slab DMA, so shuttling 4 KB of partials through the
host between launches is much cheaper):
  Launch A: per-core transposed slab W0T [n-chunk k, c, h]; matvec1 as 128
    PSUM-accumulated matmuls (lhsT = W0T chunk [k, h], rhs = x_norm column),
    fully hidden under the slab DMA. Returns V0 partials [128, 1] per core.
  Host: concatenates the 8 partials (pure gather, no arithmetic).
  Launch B: per-core natural slab W0 [h, n]; reduces partials on-device,
    runs the tiny forward/backward tanh chain on PE/ACT/DVE in column space,
    then matvec2 as 128 stationary-weight matmuls (lhsT = W0 chunk [h, k],
    rhs = r0 column) pipelined with the slab DMA. Scales by 1/x_range and
    returns the Jacobian slab [k, c] plus the scalar h.
"""

import os
import sys

import numpy as np

sys.path.insert(0, "/opt/trn_rl_repo")

import concourse.tile as tile  # noqa: E402
from concourse import bacc, mybir  # noqa: E402
from concourse import bass_utils  # noqa: E402


def _ensure_ntff_hook():
    """bass_utils' trace path imports antenv.axon_hooks, which some images
    lack (boot degrades silently and never registers the NTFF hook). Shim
    the module and register the same ctypes hook trn_boot would have."""
    try:
        import antenv.axon_hooks  # noqa: F401
        return
    except ImportError:
        pass
    import types

    mod = types.ModuleType("antenv.axon_hooks")
    _h = [None]
    mod.set_axon_ntff_profile_hook = lambda h: _h.__setitem__(0, h)
    mod.get_axon_ntff_profile_hook = lambda: _h[0]
    try:
        import antenv

        antenv.axon_hooks = mod
    except ImportError:
        pass
    sys.modules["antenv.axon_hooks"] = mod
    try:
        from trn_agent_boot.trn_boot import _ntff_profile_via_ctypes

        hook = _ntff_profile_via_ctypes("/opt/axon/libaxon_pjrt.so")
        if hook is not None:
            mod.set_axon_ntff_profile_hook(hook)
    except Exception:
        pass


_ensure_ntff_hook()

N_STATE = 131072
H = 128
N_CORES = 8
N_LOC = N_STATE // N_CORES  # 16384
C = N_LOC // 128  # 128 chunks of 128 per core
W0_TILE = 2048
N_W0_TILES = N_LOC // W0_TILE  # 8
CH_PER_TILE = W0_TILE // 128  # 16

FP = mybir.dt.float32
AOT = mybir.AluOpType
ACT = mybir.ActivationFunctionType

_CACHE = {}


def _build_a():
    """Launch A: transposed slab, matvec1 partials."""
    nc = bacc.Bacc("TRN2", target_bir_lowering=False, debug=False,
                   num_devices=N_CORES)

    w0t_d = nc.dram_tensor("w0t", [128, N_LOC], FP, kind="ExternalInput").ap()
    xsT_d = nc.dram_tensor("xsT", [128, C], FP, kind="ExternalInput").ap()
    xmaxT_d = nc.dram_tensor("xmaxT", [128, C], FP, kind="ExternalInput").ap()
    xminT_d = nc.dram_tensor("xminT", [128, C], FP, kind="ExternalInput").ap()
    outp_d = nc.dram_tensor("out_p", [H, 1], FP, kind="ExternalOutput").ap()

    with tile.TileContext(nc) as tc:
        with tc.tile_pool(name="w0", bufs=N_W0_TILES) as w0p, \
             tc.tile_pool(name="small", bufs=1) as sp, \
             tc.tile_pool(name="psl", bufs=1, space="PSUM") as plp:

            xsT = sp.tile([128, C], FP)
            nc.sync.dma_start(xsT[:], xsT_d[:])
            xmaxT = sp.tile([128, C], FP)
            nc.sync.dma_start(xmaxT[:], xmaxT_d[:])
            xminT = sp.tile([128, C], FP)
            nc.sync.dma_start(xminT[:], xminT_d[:])

            # x_norm = (state - (max+min)/2) * (2/(max-min)), in [k, c] layout
            xcT = sp.tile([128, C], FP)
            nc.vector.tensor_add(xcT[:], xmaxT[:], xminT[:])
            nc.vector.tensor_scalar_mul(xcT[:], xcT[:], 0.5)
            xrT = sp.tile([128, C], FP)
            nc.vector.tensor_sub(xrT[:], xmaxT[:], xminT[:])
            nc.vector.tensor_scalar_mul(xrT[:], xrT[:], 0.5)
            invT = sp.tile([128, C], FP)
            nc.vector.reciprocal(invT[:], xrT[:])
            xnT = sp.tile([128, C], FP)
            nc.vector.tensor_sub(xnT[:], xsT[:], xcT[:])
            nc.vector.tensor_mul(xnT[:], xnT[:], invT[:])

            v0ps = plp.tile([H, 1], FP)
            for t in range(N_W0_TILES):
                w0tile = w0p.tile([128, W0_TILE], FP, tag="w0tile")
                eng = nc.sync if t % 2 == 0 else nc.scalar
                eng.dma_start(w0tile[:], w0t_d[:, t * W0_TILE:(t + 1) * W0_TILE])
                for cc in range(CH_PER_TILE):
                    c = t * CH_PER_TILE + cc
                    nc.tensor.matmul(
                        v0ps[:],
                        w0tile[:, cc * 128:(cc + 1) * 128],
                        xnT[:, c:c + 1],
                        start=(c == 0),
                        stop=(c == C - 1),
                    )

            v0sb = sp.tile([H, 1], FP)
            nc.vector.tensor_copy(v0sb[:], v0ps[:])
            nc.sync.dma_start(outp_d[:], v0sb[:])

    nc.compile()
    return nc


def _build_b():
    """Launch B: natural slab, partial-reduce + chain + matvec2."""
    nc = bacc.Bacc("TRN2", target_bir_lowering=False, debug=False,
                   num_devices=N_CORES)

    w0_d = nc.dram_tensor("w0", [H, N_LOC], FP, kind="ExternalInput").ap()
    parts_d = nc.dram_tensor("parts", [H, N_CORES], FP, kind="ExternalInput").ap()
    xmaxT_d = nc.dram_tensor("xmaxT", [128, C], FP, kind="ExternalInput").ap()
    xminT_d = nc.dram_tensor("xminT", [128, C], FP, kind="ExternalInput").ap()
    w1t_d = nc.dram_tensor("w1t", [H, H], FP, kind="ExternalInput").ap()
    w2t_d = nc.dram_tensor("w2t", [H, H], FP, kind="ExternalInput").ap()
    w3t_d = nc.dram_tensor("w3t", [H, H], FP, kind="ExternalInput").ap()
    w1n_d = nc.dram_tensor("w1n", [H, H], FP, kind="ExternalInput").ap()
    w2n_d = nc.dram_tensor("w2n", [H, H], FP, kind="ExternalInput").ap()
    w3n_d = nc.dram_tensor("w3n", [H, H], FP, kind="ExternalInput").ap()
    # bcols columns: 0=b0 1=b1 2=b2 3=b3 4=Wout.T
    bcols_d = nc.dram_tensor("bcols", [H, 8], FP, kind="ExternalInput").ap()
    bout_d = nc.dram_tensor("bout", [1, 1], FP, kind="ExternalInput").ap()

    outj_d = nc.dram_tensor("out_j", [128, C], FP, kind="ExternalOutput").ap()
    outv_d = nc.dram_tensor("out_v", [1, 1], FP, kind="ExternalOutput").ap()

    with tile.TileContext(nc) as tc:
        with tc.tile_pool(name="w0", bufs=N_W0_TILES) as w0p, \
             tc.tile_pool(name="small", bufs=1) as sp, \
             tc.tile_pool(name="ps", bufs=2, space="PSUM") as pp, \
             tc.tile_pool(name="psj", bufs=1, space="PSUM") as pjp:

            # small loads
            parts = sp.tile([H, N_CORES], FP)
            nc.sync.dma_start(parts[:], parts_d[:])
            xmaxT = sp.tile([128, C], FP)
            nc.sync.dma_start(xmaxT[:], xmaxT_d[:])
            xminT = sp.tile([128, C], FP)
            nc.sync.dma_start(xminT[:], xminT_d[:])
            w1t = sp.tile([H, H], FP)
            nc.sync.dma_start(w1t[:], w1t_d[:])
            w2t = sp.tile([H, H], FP)
            nc.sync.dma_start(w2t[:], w2t_d[:])
            w3t = sp.tile([H, H], FP)
            nc.sync.dma_start(w3t[:], w3t_d[:])
            w1n = sp.tile([H, H], FP)
            nc.sync.dma_start(w1n[:], w1n_d[:])
            w2n = sp.tile([H, H], FP)
            nc.sync.dma_start(w2n[:], w2n_d[:])
            w3n = sp.tile([H, H], FP)
            nc.sync.dma_start(w3n[:], w3n_d[:])
            bcols = sp.tile([H, 8], FP)
            nc.sync.dma_start(bcols[:], bcols_d[:])
            boutt = sp.tile([1, 1], FP)
            nc.sync.dma_start(boutt[:], bout_d[:])
            one11 = sp.tile([1, 1], FP)
            nc.vector.memset(one11[:], 1.0)

            # 1/x_range in [k, c] layout
            xrT = sp.tile([128, C], FP)
            nc.vector.tensor_sub(xrT[:], xmaxT[:], xminT[:])
            nc.vector.tensor_scalar_mul(xrT[:], xrT[:], 0.5)
            invT = sp.tile([128, C], FP)
            nc.vector.reciprocal(invT[:], xrT[:])

            # ---- chain (all vectors as [128, 1] columns) ----
            v0c = sp.tile([H, 1], FP)
            nc.vector.tensor_reduce(v0c[:], parts[:], mybir.AxisListType.X, AOT.add)

            v1c = sp.tile([H, 1], FP)
            nc.scalar.activation(v1c[:], v0c[:], ACT.Tanh, bias=bcols[:, 0:1])
            d0c = sp.tile([H, 1], FP)
            nc.vector.tensor_mul(d0c[:], v1c[:], v1c[:])
            nc.vector.tensor_scalar(d0c[:], d0c[:], -1.0, 1.0, AOT.mult, AOT.add)

            a1ps = pp.tile([H, 1], FP, tag="chain_ps")
            nc.tensor.matmul(a1ps[:], w1t[:], v1c[:], start=True, stop=True)
            v2c = sp.tile([H, 1], FP)
            nc.scalar.activation(v2c[:], a1ps[:], ACT.Tanh, bias=bcols[:, 1:2])
            d1c = sp.tile([H, 1], FP)
            nc.vector.tensor_mul(d1c[:], v2c[:], v2c[:])
            nc.vector.tensor_scalar(d1c[:], d1c[:], -1.0, 1.0, AOT.mult, AOT.add)

            a2ps = pp.tile([H, 1], FP, tag="chain_ps")
            nc.tensor.matmul(a2ps[:], w2t[:], v2c[:], start=True, stop=True)
            v3c = sp.tile([H, 1], FP)
            nc.scalar.activation(v3c[:], a2ps[:], ACT.Tanh, bias=bcols[:, 2:3])
            d2c = sp.tile([H, 1], FP)
            nc.vector.tensor_mul(d2c[:], v3c[:], v3c[:])
            nc.vector.tensor_scalar(d2c[:], d2c[:], -1.0, 1.0, AOT.mult, AOT.add)

            a3ps = pp.tile([H, 1], FP, tag="chain_ps")
            nc.tensor.matmul(a3ps[:], w3t[:], v3c[:], start=True, stop=True)
            a3c = sp.tile([H, 1], FP)
            nc.scalar.activation(a3c[:], a3ps[:], ACT.Identity, bias=bcols[:, 3:4])

            voutps = pp.tile([1, 1], FP, tag="chain_ps")
            nc.tensor.matmul(voutps[:], bcols[:, 4:5], a3c[:], start=True, stop=True)
            voutsb = sp.tile([1, 1], FP)
            nc.scalar.activation(voutsb[:], voutps[:], ACT.Identity, bias=boutt[:])
            nc.sync.dma_start(outv_d[:], voutsb[:])

            # backward: r3 = Wout @ W3; q2 = r3*d2; r2 = q2 @ W2; q1 = r2*d1;
            # r1 = q1 @ W1; r0 = r1*d0
            r3rp = pp.tile([1, H], FP, tag="chain_ps")
            nc.tensor.matmul(r3rp[:], bcols[:, 4:5], w3n[:], start=True, stop=True)
            r3r = sp.tile([1, H], FP)
            nc.vector.tensor_copy(r3r[:], r3rp[:])
            r3cp = pp.tile([H, 1], FP, tag="chain_ps")
            nc.tensor.matmul(r3cp[:], r3r[:], one11[:], start=True, stop=True)
            q2c = sp.tile([H, 1], FP)
            nc.vector.tensor_mul(q2c[:], r3cp[:], d2c[:])

            r2rp = pp.tile([1, H], FP, tag="chain_ps")
            nc.tensor.matmul(r2rp[:], q2c[:], w2n[:], start=True, stop=True)
            r2r = sp.tile([1, H], FP)
            nc.vector.tensor_copy(r2r[:], r2rp[:])
            r2cp = pp.tile([H, 1], FP, tag="chain_ps")
            nc.tensor.matmul(r2cp[:], r2r[:], one11[:], start=True, stop=True)
            q1c = sp.tile([H, 1], FP)
            nc.vector.tensor_mul(q1c[:], r2cp[:], d1c[:])

            r1rp = pp.tile([1, H], FP, tag="chain_ps")
            nc.tensor.matmul(r1rp[:], q1c[:], w1n[:], start=True, stop=True)
            r1r = sp.tile([1, H], FP)
            nc.vector.tensor_copy(r1r[:], r1rp[:])
            r1cp = pp.tile([H, 1], FP, tag="chain_ps")
            nc.tensor.matmul(r1cp[:], r1r[:], one11[:], start=True, stop=True)
            r0c = sp.tile([H, 1], FP)
            nc.vector.tensor_mul(r0c[:], r1cp[:], d0c[:])

            # ---- matvec2: J[c*128+k] = sum_h W0[h, c*128+k] * r0[h] ----
            jt = pjp.tile([128, C], FP)
            for t in range(N_W0_TILES):
                w0tile = w0p.tile([128, W0_TILE], FP, tag="w0tile")
                eng = nc.sync if t % 2 == 0 else nc.scalar
                eng.dma_start(w0tile[:], w0_d[:, t * W0_TILE:(t + 1) * W0_TILE])
                for cc in range(CH_PER_TILE):
                    c = t * CH_PER_TILE + cc
                    nc.tensor.matmul(
                        jt[:, c:c + 1],
                        w0tile[:, cc * 128:(cc + 1) * 128],
                        r0c[:],
                        start=True,
                        stop=True,
                    )

            jts = sp.tile([128, C], FP)
            nc.vector.tensor_mul(jts[:], jt[:], invT[:])
            nc.sync.dma_start(outj_d[:], jts[:])

    nc.compile()
    return nc


def _get_kernels():
    if "nc_a" not in _CACHE:
        _CACHE["nc_a"] = _build_a()
        _CACHE["nc_b"] = _build_b()
    return _CACHE["nc_a"], _CACHE["nc_b"]


def kernel(**inputs):
    nc_a, nc_b = _get_kernels()
    f = np.float32

    state = np.asarray(inputs["state"], f).reshape(1, N_STATE)
    x_max = np.asarray(inputs["x_max"], f).reshape(N_STATE)
    x_min = np.asarray(inputs["x_min"], f).reshape(N_STATE)
    W0 = np.asarray(inputs["W0"], f)
    W1 = np.asarray(inputs["W1"], f)
    W2 = np.asarray(inputs["W2"], f)
    W3 = np.asarray(inputs["W3"], f)
    Wout = np.asarray(inputs["Wout"], f).reshape(1, H)
    b0 = np.asarray(inputs["b0"], f).reshape(H)
    b1 = np.asarray(inputs["b1"], f).reshape(H)
    b2 = np.asarray(inputs["b2"], f).reshape(H)
    b3 = np.asarray(inputs["b3"], f).reshape(H)
    bout = np.asarray(inputs["bout"], f).reshape(1)

    xmaxT = []
    xminT = []
    in_maps_a = []
    for i in range(N_CORES):
        sl = slice(i * N_LOC, (i + 1) * N_LOC)
        w0t = np.ascontiguousarray(
            W0[:, sl].reshape(H, C, 128).transpose(2, 1, 0)
        ).reshape(128, C * H)
        xmaxT.append(np.ascontiguousarray(x_max[sl].reshape(C, 128).T))
        xminT.append(np.ascontiguousarray(x_min[sl].reshape(C, 128).T))
        in_maps_a.append({
            "w0t": w0t,
            "xsT": np.ascontiguousarray(state[0, sl].reshape(C, 128).T),
            "xmaxT": xmaxT[i],
            "xminT": xminT[i],
        })

    trace = bool(int(os.environ.get("KERNEL_TRACE", "0")))
    res_a = bass_utils.run_bass_kernel_spmd(
        nc_a, in_maps_a, core_ids=list(range(N_CORES)), trace=trace
    )
    _CACHE["res_a"] = res_a

    # pure gather: stack the 8 per-core partial columns
    parts = np.ascontiguousarray(
        np.concatenate([res_a.results[i]["out_p"] for i in range(N_CORES)], axis=1)
    )

    bcolsm = np.zeros((H, 8), f)
    bcolsm[:, 0] = b0
    bcolsm[:, 1] = b1
    bcolsm[:, 2] = b2
    bcolsm[:, 3] = b3
    bcolsm[:, 4] = Wout[0]
    shared_b = {
        "parts": parts,
        "w1t": np.ascontiguousarray(W1.T),
        "w2t": np.ascontiguousarray(W2.T),
        "w3t": np.ascontiguousarray(W3.T),
        "w1n": np.ascontiguousarray(W1),
        "w2n": np.ascontiguousarray(W2),
        "w3n": np.ascontiguousarray(W3),
        "bcols": bcolsm,
        "bout": np.ascontiguousarray(bout.reshape(1, 1)),
    }
    in_maps_b = []
    for i in range(N_CORES):
        sl = slice(i * N_LOC, (i + 1) * N_LOC)
        m = {
            "w0": np.ascontiguousarray(W0[:, sl]),
            "xmaxT": xmaxT[i],
            "xminT": xminT[i],
        }
        m.update(shared_b)
        in_maps_b.append(m)

    res_b = bass_utils.run_bass_kernel_spmd(
        nc_b, in_maps_b, core_ids=list(range(N_CORES)), trace=trace
    )
    _CACHE["res_b"] = res_b

    out = np.empty((1, N_STATE + 1), np.float32)
    out[0, 0] = float(np.asarray(res_b.results[0]["out_v"]).reshape(()))
    for i in range(N_CORES):
        jt = np.asarray(res_b.results[i]["out_j"])  # [k, c]
        out[0, 1 + i * N_LOC:1 + (i + 1) * N_LOC] = jt.T.reshape(-1)
    return out



# revision 3
# speedup vs baseline: 2.3430x; 2.3430x over previous
"""Trainium2 Bass kernel for nn_CBF (dense MLP forward + Jacobian row).

Math: the reference computes HJH = [h, Jh] where
  h  = MLP(x_norm)            (scalar)
  Jh = Wout @ W3 @ D2 @ W2 @ D1 @ W1 @ D0 @ (W0 / x_range)   (1 x n row)
Everything is linear in the Jacobian chain, so Jh collapses to
  r0 @ W0 / x_range   with  r0 = left-chain of tiny 128-dim vectors.
Only two passes over the big W0 (128 x 131072) are needed:
  matvec1: V0 = x_norm @ W0.T    (contraction over n)
  matvec2: J  = r0 @ W0          (contraction over h)
W0 is sharded along n over 8 cores; the big slabs are cast to fp16 on
the host (rel tolerance is 2e-2; fp16 keeps the end-to-end error ~1e-3)
which halves HBM traffic and runs the PE in single-pass 16-bit mode
(fp32 matmuls lower to 2x(LDWEIGHTS+MATMUL) and disable fast weight
load — that made the fp32 baseline PE-bound at ~57us/launch).

Two kernel launches (a cross-core AllReduce measured ~114 us on this
stack, vs ~14 us for an fp16 slab DMA, so shuttling 512 B of partials
through the host between launches is much cheaper):
  Launch A: per-core transposed slab w0t [n-chunk k, c*h]; matvec1 with
    the x-norm column as the STATIONARY operand (1-col LDWEIGHTS) and
    the W0T chunk streaming as rhs, accumulating [1,128] in PSUM —
    ~56 ns/chunk warm, MM-bound, immune to weight-load rate.
  Host: stacks the 8 per-core [1,128] partials (pure gather).
  Launch B: per-core natural slab w0 [h, n]; reduces partials
    on-device, runs the tiny fp32 forward/backward tanh chain, then
    matvec2 with W0 chunks as stationary weights (fp16 FWL) producing
    J in [k, c] layout, scaled by 1/x_range on the way out.
Both launches issue ~70 junk matmuls with no DMA dependency first so
the PE HAM clock-gate reaches 8/8 before the real matmul stream.
"""

import os
import sys

import numpy as np

sys.path.insert(0, "/opt/trn_rl_repo")

import concourse.tile as tile  # noqa: E402
from concourse import bacc, mybir  # noqa: E402
from concourse import bass_utils  # noqa: E402


def _ensure_ntff_hook():
    """bass_utils' trace path imports antenv.axon_hooks, which some images
    lack (boot degrades silently and never registers the NTFF hook). Shim
    the module and register the same ctypes hook trn_boot would have."""
    try:
        import antenv.axon_hooks  # noqa: F401
        return
    except ImportError:
        pass
    import types

    mod = types.ModuleType("antenv.axon_hooks")
    _h = [None]
    mod.set_axon_ntff_profile_hook = lambda h: _h.__setitem__(0, h)
    mod.get_axon_ntff_profile_hook = lambda: _h[0]
    try:
        import antenv

        antenv.axon_hooks = mod
    except ImportError:
        pass
    sys.modules["antenv.axon_hooks"] = mod
    try:
        from trn_agent_boot.trn_boot import _ntff_profile_via_ctypes

        hook = _ntff_profile_via_ctypes("/opt/axon/libaxon_pjrt.so")
        if hook is not None:
            mod.set_axon_ntff_profile_hook(hook)
    except Exception:
        pass


_ensure_ntff_hook()

N_STATE = 131072
H = 128
N_CORES = 8
N_LOC = N_STATE // N_CORES  # 16384
C = N_LOC // 128  # 128 chunks of 128 per core
W0_TILE = 2048
N_W0_TILES = N_LOC // W0_TILE  # 8
CH_PER_TILE = W0_TILE // 128  # 16
N_WARM = 72  # junk matmuls to warm the PE HAM clock gate (~3.8us cold)

FP = mybir.dt.float32
F16 = mybir.dt.float16
AOT = mybir.AluOpType
ACT = mybir.ActivationFunctionType

_CACHE = {}


def _warmup_pe(nc, sp, pwp):
    """Junk matmuls with no DMA dependency: run during the DMA ramp so the
    PE HAM clock gate is at 8/8 when the real matmul stream starts."""
    junk = sp.tile([128, 64], F16)
    nc.vector.memset(junk[:], 0.0)
    wps = pwp.tile([1, 64], FP)
    for _ in range(N_WARM):
        nc.tensor.matmul(wps[:], junk[:, 0:1], junk[:], start=True, stop=True)


def _build_a():
    """Launch A: transposed fp16 slab, matvec1 partial [1,128]."""
    nc = bacc.Bacc("TRN2", target_bir_lowering=False, debug=False,
                   num_devices=N_CORES)

    w0t_d = nc.dram_tensor("w0t", [128, N_LOC], F16, kind="ExternalInput").ap()
    xsT_d = nc.dram_tensor("xsT", [128, C], FP, kind="ExternalInput").ap()
    xmaxT_d = nc.dram_tensor("xmaxT", [128, C], FP, kind="ExternalInput").ap()
    xminT_d = nc.dram_tensor("xminT", [128, C], FP, kind="ExternalInput").ap()
    outp_d = nc.dram_tensor("out_p", [1, H], FP, kind="ExternalOutput").ap()

    with tile.TileContext(nc) as tc:
        with tc.tile_pool(name="w0", bufs=N_W0_TILES) as w0p, \
             tc.tile_pool(name="small", bufs=1) as sp, \
             tc.tile_pool(name="psl", bufs=1, space="PSUM") as plp, \
             tc.tile_pool(name="psw", bufs=1, space="PSUM") as pwp:

            _warmup_pe(nc, sp, pwp)

            xsT = sp.tile([128, C], FP)
            nc.gpsimd.dma_start(xsT[:], xsT_d[:])
            xmaxT = sp.tile([128, C], FP)
            nc.gpsimd.dma_start(xmaxT[:], xmaxT_d[:])
            xminT = sp.tile([128, C], FP)
            nc.gpsimd.dma_start(xminT[:], xminT_d[:])

            # x_norm = (state - (max+min)/2) * (2/(max-min)), in [k, c] layout
            xcT = sp.tile([128, C], FP)
            nc.vector.tensor_add(xcT[:], xmaxT[:], xminT[:])
            nc.vector.tensor_scalar_mul(xcT[:], xcT[:], 0.5)
            xrT = sp.tile([128, C], FP)
            nc.vector.tensor_sub(xrT[:], xmaxT[:], xminT[:])
            nc.vector.tensor_scalar_mul(xrT[:], xrT[:], 0.5)
            invT = sp.tile([128, C], FP)
            nc.vector.reciprocal(invT[:], xrT[:])
            xnT = sp.tile([128, C], FP)
            nc.vector.tensor_sub(xnT[:], xsT[:], xcT[:])
            nc.vector.tensor_mul(xnT[:], xnT[:], invT[:])
            xn16 = sp.tile([128, C], F16)
            nc.vector.tensor_copy(xn16[:], xnT[:])

            # matvec1: V0[h] += sum_k x[k,c] * W0T[k, c, h]; the x column is
            # the stationary operand (1-col LDWEIGHTS), W0T streams as rhs.
            v0ps = plp.tile([1, H], FP)
            for t in range(N_W0_TILES):
                w0tile = w0p.tile([128, W0_TILE], F16, tag="w0tile")
                eng = nc.sync if t % 2 == 0 else nc.scalar
                eng.dma_start(w0tile[:], w0t_d[:, t * W0_TILE:(t + 1) * W0_TILE])
                for cc in range(CH_PER_TILE):
                    c = t * CH_PER_TILE + cc
                    nc.tensor.matmul(
                        v0ps[:],
                        xn16[:, c:c + 1],
                        w0tile[:, cc * 128:(cc + 1) * 128],
                        start=(c == 0),
                        stop=(c == C - 1),
                    )

            v0sb = sp.tile([1, H], FP)
            nc.vector.tensor_copy(v0sb[:], v0ps[:])
            nc.sync.dma_start(outp_d[:], v0sb[:])

    nc.compile()
    return nc


def _build_b():
    """Launch B: natural fp16 slab, partial-reduce + chain + matvec2."""
    nc = bacc.Bacc("TRN2", target_bir_lowering=False, debug=False,
                   num_devices=N_CORES)

    w0_d = nc.dram_tensor("w0", [H, N_LOC], F16, kind="ExternalInput").ap()
    parts_d = nc.dram_tensor("parts", [H, N_CORES], FP, kind="ExternalInput").ap()
    xmaxT_d = nc.dram_tensor("xmaxT", [128, C], FP, kind="ExternalInput").ap()
    xminT_d = nc.dram_tensor("xminT", [128, C], FP, kind="ExternalInput").ap()
    w1t_d = nc.dram_tensor("w1t", [H, H], FP, kind="ExternalInput").ap()
    w2t_d = nc.dram_tensor("w2t", [H, H], FP, kind="ExternalInput").ap()
    w3t_d = nc.dram_tensor("w3t", [H, H], FP, kind="ExternalInput").ap()
    w1n_d = nc.dram_tensor("w1n", [H, H], FP, kind="ExternalInput").ap()
    w2n_d = nc.dram_tensor("w2n", [H, H], FP, kind="ExternalInput").ap()
    w3n_d = nc.dram_tensor("w3n", [H, H], FP, kind="ExternalInput").ap()
    # bcols columns: 0=b0 1=b1 2=b2 3=b3 4=Wout.T
    bcols_d = nc.dram_tensor("bcols", [H, 8], FP, kind="ExternalInput").ap()
    bout_d = nc.dram_tensor("bout", [1, 1], FP, kind="ExternalInput").ap()

    outj_d = nc.dram_tensor("out_j", [128, C], FP, kind="ExternalOutput").ap()
    outv_d = nc.dram_tensor("out_v", [1, 1], FP, kind="ExternalOutput").ap()

    with tile.TileContext(nc) as tc:
        with tc.tile_pool(name="w0", bufs=N_W0_TILES) as w0p, \
             tc.tile_pool(name="small", bufs=1) as sp, \
             tc.tile_pool(name="ps", bufs=2, space="PSUM") as pp, \
             tc.tile_pool(name="psj", bufs=1, space="PSUM") as pjp, \
             tc.tile_pool(name="psw", bufs=1, space="PSUM") as pwp:

            _warmup_pe(nc, sp, pwp)

            # small loads on the gpsimd queue so the big w0 tiles own
            # the sync/scalar queues from the start
            parts = sp.tile([H, N_CORES], FP)
            nc.gpsimd.dma_start(parts[:], parts_d[:])
            xmaxT = sp.tile([128, C], FP)
            nc.gpsimd.dma_start(xmaxT[:], xmaxT_d[:])
            xminT = sp.tile([128, C], FP)
            nc.gpsimd.dma_start(xminT[:], xminT_d[:])
            w1t = sp.tile([H, H], FP)
            nc.gpsimd.dma_start(w1t[:], w1t_d[:])
            w2t = sp.tile([H, H], FP)
            nc.gpsimd.dma_start(w2t[:], w2t_d[:])
            w3t = sp.tile([H, H], FP)
            nc.gpsimd.dma_start(w3t[:], w3t_d[:])
            w1n = sp.tile([H, H], FP)
            nc.gpsimd.dma_start(w1n[:], w1n_d[:])
            w2n = sp.tile([H, H], FP)
            nc.gpsimd.dma_start(w2n[:], w2n_d[:])
            w3n = sp.tile([H, H], FP)
            nc.gpsimd.dma_start(w3n[:], w3n_d[:])
            bcols = sp.tile([H, 8], FP)
            nc.gpsimd.dma_start(bcols[:], bcols_d[:])
            boutt = sp.tile([1, 1], FP)
            nc.gpsimd.dma_start(boutt[:], bout_d[:])
            one11 = sp.tile([1, 1], FP)
            nc.vector.memset(one11[:], 1.0)

            # 1/x_range in [k, c] layout
            xrT = sp.tile([128, C], FP)
            nc.vector.tensor_sub(xrT[:], xmaxT[:], xminT[:])
            nc.vector.tensor_scalar_mul(xrT[:], xrT[:], 0.5)
            invT = sp.tile([128, C], FP)
            nc.vector.reciprocal(invT[:], xrT[:])

            # ---- chain (all vectors as [128, 1] columns) ----
            v0c = sp.tile([H, 1], FP)
            nc.vector.tensor_reduce(v0c[:], parts[:], mybir.AxisListType.X, AOT.add)

            v1c = sp.tile([H, 1], FP)
            nc.scalar.activation(v1c[:], v0c[:], ACT.Tanh, bias=bcols[:, 0:1])
            d0c = sp.tile([H, 1], FP)
            nc.vector.tensor_mul(d0c[:], v1c[:], v1c[:])
            nc.vector.tensor_scalar(d0c[:], d0c[:], -1.0, 1.0, AOT.mult, AOT.add)

            a1ps = pp.tile([H, 1], FP, tag="chain_ps")
            nc.tensor.matmul(a1ps[:], w1t[:], v1c[:], start=True, stop=True)
            v2c = sp.tile([H, 1], FP)
            nc.scalar.activation(v2c[:], a1ps[:], ACT.Tanh, bias=bcols[:, 1:2])
            d1c = sp.tile([H, 1], FP)
            nc.vector.tensor_mul(d1c[:], v2c[:], v2c[:])
            nc.vector.tensor_scalar(d1c[:], d1c[:], -1.0, 1.0, AOT.mult, AOT.add)

            a2ps = pp.tile([H, 1], FP, tag="chain_ps")
            nc.tensor.matmul(a2ps[:], w2t[:], v2c[:], start=True, stop=True)
            v3c = sp.tile([H, 1], FP)
            nc.scalar.activation(v3c[:], a2ps[:], ACT.Tanh, bias=bcols[:, 2:3])
            d2c = sp.tile([H, 1], FP)
            nc.vector.tensor_mul(d2c[:], v3c[:], v3c[:])
            nc.vector.tensor_scalar(d2c[:], d2c[:], -1.0, 1.0, AOT.mult, AOT.add)

            a3ps = pp.tile([H, 1], FP, tag="chain_ps")
            nc.tensor.matmul(a3ps[:], w3t[:], v3c[:], start=True, stop=True)
            a3c = sp.tile([H, 1], FP)
            nc.scalar.activation(a3c[:], a3ps[:], ACT.Identity, bias=bcols[:, 3:4])

            voutps = pp.tile([1, 1], FP, tag="chain_ps")
            nc.tensor.matmul(voutps[:], bcols[:, 4:5], a3c[:], start=True, stop=True)
            voutsb = sp.tile([1, 1], FP)
            nc.scalar.activation(voutsb[:], voutps[:], ACT.Identity, bias=boutt[:])
            nc.sync.dma_start(outv_d[:], voutsb[:])

            # backward: r3 = Wout @ W3; q2 = r3*d2; r2 = q2 @ W2; q1 = r2*d1;
            # r1 = q1 @ W1; r0 = r1*d0
            r3rp = pp.tile([1, H], FP, tag="chain_ps")
            nc.tensor.matmul(r3rp[:], bcols[:, 4:5], w3n[:], start=True, stop=True)
            r3r = sp.tile([1, H], FP)
            nc.vector.tensor_copy(r3r[:], r3rp[:])
            r3cp = pp.tile([H, 1], FP, tag="chain_ps")
            nc.tensor.matmul(r3cp[:], r3r[:], one11[:], start=True, stop=True)
            q2c = sp.tile([H, 1], FP)
            nc.vector.tensor_mul(q2c[:], r3cp[:], d2c[:])

            r2rp = pp.tile([1, H], FP, tag="chain_ps")
            nc.tensor.matmul(r2rp[:], q2c[:], w2n[:], start=True, stop=True)
            r2r = sp.tile([1, H], FP)
            nc.vector.tensor_copy(r2r[:], r2rp[:])
            r2cp = pp.tile([H, 1], FP, tag="chain_ps")
            nc.tensor.matmul(r2cp[:], r2r[:], one11[:], start=True, stop=True)
            q1c = sp.tile([H, 1], FP)
            nc.vector.tensor_mul(q1c[:], r2cp[:], d1c[:])

            r1rp = pp.tile([1, H], FP, tag="chain_ps")
            nc.tensor.matmul(r1rp[:], q1c[:], w1n[:], start=True, stop=True)
            r1r = sp.tile([1, H], FP)
            nc.vector.tensor_copy(r1r[:], r1rp[:])
            r1cp = pp.tile([H, 1], FP, tag="chain_ps")
            nc.tensor.matmul(r1cp[:], r1r[:], one11[:], start=True, stop=True)
            r0c = sp.tile([H, 1], FP)
            nc.vector.tensor_mul(r0c[:], r1cp[:], d0c[:])
            r016 = sp.tile([H, 1], F16)
            nc.vector.tensor_copy(r016[:], r0c[:])

            # ---- matvec2: J[c*128+k] = sum_h W0[h, c*128+k] * r0[h] ----
            jt = pjp.tile([128, C], FP)
            for t in range(N_W0_TILES):
                w0tile = w0p.tile([128, W0_TILE], F16, tag="w0tile")
                eng = nc.sync if t % 2 == 0 else nc.scalar
                eng.dma_start(w0tile[:], w0_d[:, t * W0_TILE:(t + 1) * W0_TILE])
                for cc in range(CH_PER_TILE):
                    c = t * CH_PER_TILE + cc
                    nc.tensor.matmul(
                        jt[:, c:c + 1],
                        w0tile[:, cc * 128:(cc + 1) * 128],
                        r016[:],
                        start=True,
                        stop=True,
                    )

            jts = sp.tile([128, C], FP)
            nc.vector.tensor_mul(jts[:], jt[:], invT[:])
            nc.sync.dma_start(outj_d[:], jts[:])

    nc.compile()
    return nc


def _get_kernels():
    if "nc_a" not in _CACHE:
        _CACHE["nc_a"] = _build_a()
        _CACHE["nc_b"] = _build_b()
    return _CACHE["nc_a"], _CACHE["nc_b"]


def kernel(**inputs):
    nc_a, nc_b = _get_kernels()
    f = np.float32

    state = np.asarray(inputs["state"], f).reshape(1, N_STATE)
    x_max = np.asarray(inputs["x_max"], f).reshape(N_STATE)
    x_min = np.asarray(inputs["x_min"], f).reshape(N_STATE)
    W0 = np.asarray(inputs["W0"], f)
    W1 = np.asarray(inputs["W1"], f)
    W2 = np.asarray(inputs["W2"], f)
    W3 = np.asarray(inputs["W3"], f)
    Wout = np.asarray(inputs["Wout"], f).reshape(1, H)
    b0 = np.asarray(inputs["b0"], f).reshape(H)
    b1 = np.asarray(inputs["b1"], f).reshape(H)
    b2 = np.asarray(inputs["b2"], f).reshape(H)
    b3 = np.asarray(inputs["b3"], f).reshape(H)
    bout = np.asarray(inputs["bout"], f).reshape(1)

    W016 = W0.astype(np.float16)

    xmaxT = []
    xminT = []
    in_maps_a = []
    for i in range(N_CORES):
        sl = slice(i * N_LOC, (i + 1) * N_LOC)
        w0t = np.ascontiguousarray(
            W016[:, sl].reshape(H, C, 128).transpose(2, 1, 0)
        ).reshape(128, C * H)
        xmaxT.append(np.ascontiguousarray(x_max[sl].reshape(C, 128).T))
        xminT.append(np.ascontiguousarray(x_min[sl].reshape(C, 128).T))
        in_maps_a.append({
            "w0t": w0t,
            "xsT": np.ascontiguousarray(state[0, sl].reshape(C, 128).T),
            "xmaxT": xmaxT[i],
            "xminT": xminT[i],
        })

    trace = bool(int(os.environ.get("KERNEL_TRACE", "0")))
    res_a = bass_utils.run_bass_kernel_spmd(
        nc_a, in_maps_a, core_ids=list(range(N_CORES)), trace=trace
    )
    _CACHE["res_a"] = res_a

    # pure gather: stack the 8 per-core partial rows into [H, 8] columns
    parts = np.ascontiguousarray(
        np.stack([res_a.results[i]["out_p"][0] for i in range(N_CORES)], axis=1)
    )

    bcolsm = np.zeros((H, 8), f)
    bcolsm[:, 0] = b0
    bcolsm[:, 1] = b1
    bcolsm[:, 2] = b2
    bcolsm[:, 3] = b3
    bcolsm[:, 4] = Wout[0]
    shared_b = {
        "parts": parts,
        "w1t": np.ascontiguousarray(W1.T),
        "w2t": np.ascontiguousarray(W2.T),
        "w3t": np.ascontiguousarray(W3.T),
        "w1n": np.ascontiguousarray(W1),
        "w2n": np.ascontiguousarray(W2),
        "w3n": np.ascontiguousarray(W3),
        "bcols": bcolsm,
        "bout": np.ascontiguousarray(bout.reshape(1, 1)),
    }
    in_maps_b = []
    for i in range(N_CORES):
        sl = slice(i * N_LOC, (i + 1) * N_LOC)
        m = {
            "w0": np.ascontiguousarray(W016[:, sl]),
            "xmaxT": xmaxT[i],
            "xminT": xminT[i],
        }
        m.update(shared_b)
        in_maps_b.append(m)

    res_b = bass_utils.run_bass_kernel_spmd(
        nc_b, in_maps_b, core_ids=list(range(N_CORES)), trace=trace
    )
    _CACHE["res_b"] = res_b

    out = np.empty((1, N_STATE + 1), np.float32)
    out[0, 0] = float(np.asarray(res_b.results[0]["out_v"]).reshape(()))
    for i in range(N_CORES):
        jt = np.asarray(res_b.results[i]["out_j"])  # [k, c]
        out[0, 1 + i * N_LOC:1 + (i + 1) * N_LOC] = jt.T.reshape(-1)
    return out


# revision 6
# speedup vs baseline: 2.8408x; 1.2125x over previous
"""Trainium2 Bass kernel for nn_CBF (dense MLP forward + Jacobian row).

Math: the reference computes HJH = [h, Jh] where
  h  = MLP(x_norm)            (scalar)
  Jh = Wout @ W3 @ D2 @ W2 @ D1 @ W1 @ D0 @ (W0 / x_range)   (1 x n row)
Everything is linear in the Jacobian chain, so Jh collapses to
  r0 @ W0 / x_range   with  r0 = left-chain of tiny 128-dim vectors.
Only two passes over the big W0 (128 x 131072) are needed:
  matvec1: V0 = x_norm @ W0.T    (contraction over n)
  matvec2: J  = r0 @ W0          (contraction over h)
W0 is sharded along n over 8 cores; the big slabs are cast to fp16 on
the host (rel tolerance is 2e-2; fp16 keeps the end-to-end error ~1e-3)
which halves HBM traffic and runs the PE in single-pass 16-bit mode.

Two kernel launches (a cross-core AllReduce measured ~114 us on this
stack vs ~12 us for an fp16 slab DMA, so shuttling 512 B of partials
through the host between launches is much cheaper):
  Launch A: per-core transposed slab w0t [n-chunk k, c*h] streamed over
    4 DMA queues; matvec1 with the x-norm column (host-precomputed fp16)
    as the STATIONARY operand and the W0T chunk streaming as rhs,
    accumulating [1,128] in PSUM — ~72 ns/chunk warm, MM-bound.
  Host: stacks the 8 per-core [1,128] partials (pure gather).
  Launch B: per-core natural slab w0 [h, n] over 4 queues; all small
    operands ride ONE packed [128,S] fp32 tensor that streams ahead of
    the slab (v2's separate small DMAs didn't land until t=22us and
    serialized the whole launch behind them); fp32 forward/backward tanh
    chain with direct column-matmuls for the backward pass; matvec2 with
    W0 chunks as stationary weights (fp16 FWL, ~30 ns/chunk) producing
    J in [k, c] layout, scaled by host-precomputed 1/x_range, stored in
    two overlapped halves.
Both launches issue junk matmuls with no DMA dependency first so the PE
HAM clock-gate reaches 8/8 before the real matmul stream.
"""

import os
import sys

import numpy as np

sys.path.insert(0, "/opt/trn_rl_repo")

import concourse.tile as tile  # noqa: E402
from concourse import bacc, mybir  # noqa: E402
from concourse import bass_utils  # noqa: E402


def _ensure_ntff_hook():
    """bass_utils' trace path imports antenv.axon_hooks, which some images
    lack (boot degrades silently and never registers the NTFF hook). Shim
    the module and register the same ctypes hook trn_boot would have."""
    try:
        import antenv.axon_hooks  # noqa: F401
        return
    except ImportError:
        pass
    import types

    mod = types.ModuleType("antenv.axon_hooks")
    _h = [None]
    mod.set_axon_ntff_profile_hook = lambda h: _h.__setitem__(0, h)
    mod.get_axon_ntff_profile_hook = lambda: _h[0]
    try:
        import antenv

        antenv.axon_hooks = mod
    except ImportError:
        pass
    sys.modules["antenv.axon_hooks"] = mod
    try:
        from trn_agent_boot.trn_boot import _ntff_profile_via_ctypes

        hook = _ntff_profile_via_ctypes("/opt/axon/libaxon_pjrt.so")
        if hook is not None:
            mod.set_axon_ntff_profile_hook(hook)
    except Exception:
        pass


_ensure_ntff_hook()

N_STATE = 131072
H = 128
N_CORES = 8
N_LOC = N_STATE // N_CORES  # 16384
C = N_LOC // 128  # 128 chunks of 128 per core
W0_TILE = 2048
N_W0_TILES = N_LOC // W0_TILE  # 8
CH_PER_TILE = W0_TILE // 128  # 16
N_WARM = 64  # junk matmuls to warm the PE HAM clock gate

# packed-smalls column layout for launch B (fp32 [128, SB])
SB_PARTS = 0        # 8 cols
SB_W1T = 8          # W1.T
SB_W2T = 136        # W2.T
SB_W3T = 264        # W3.T
SB_W1N = 392        # W1
SB_W2N = 520        # W2
SB_W3N = 648        # W3
SB_BCOL = 776       # 8 cols: b0 b1 b2 b3 woutT bout - -
SB_INVT = 784       # 1/x_range in [k, c] layout, 128 cols
SB = 912

FP = mybir.dt.float32
F16 = mybir.dt.float16
AOT = mybir.AluOpType
ACT = mybir.ActivationFunctionType

_CACHE = {}


def _warmup_pe(nc, sp, pwp):
    """Junk matmuls with no DMA dependency: run during the DMA ramp so the
    PE HAM clock gate is at 8/8 when the real matmul stream starts."""
    junk = sp.tile([128, 64], F16)
    nc.vector.memset(junk[:], 0.0)
    wps = pwp.tile([1, 64], FP)
    for _ in range(N_WARM):
        nc.tensor.matmul(wps[:], junk[:, 0:1], junk[:], start=True, stop=True)


_QUEUES = ["sync", "scalar", "gpsimd"]


def _build_a():
    """Launch A: transposed fp16 slab, matvec1 partial [1,128]."""
    nc = bacc.Bacc("TRN2", target_bir_lowering=False, debug=False,
                   num_devices=N_CORES)

    w0t_d = nc.dram_tensor("w0t", [128, N_LOC], F16, kind="ExternalInput").ap()
    xn_d = nc.dram_tensor("xn16", [128, C], F16, kind="ExternalInput").ap()
    outp_d = nc.dram_tensor("out_p", [1, H], FP, kind="ExternalOutput").ap()

    with tile.TileContext(nc) as tc:
        with tc.tile_pool(name="w0", bufs=N_W0_TILES) as w0p, \
             tc.tile_pool(name="small", bufs=1) as sp, \
             tc.tile_pool(name="psl", bufs=1, space="PSUM") as plp, \
             tc.tile_pool(name="psw", bufs=1, space="PSUM") as pwp:

            _warmup_pe(nc, sp, pwp)

            xn16 = sp.tile([128, C], F16)
            nc.sync.dma_start(xn16[:], xn_d[:])

            # matvec1: V0[h] += sum_k x[k,c] * W0T[k, c, h]; the x column is
            # the stationary operand (1-col LDWEIGHTS), W0T streams as rhs.
            v0ps = plp.tile([1, H], FP)
            for t in range(N_W0_TILES):
                w0tile = w0p.tile([128, W0_TILE], F16, tag="w0tile")
                eng = getattr(nc, _QUEUES[t % 3])
                eng.dma_start(w0tile[:], w0t_d[:, t * W0_TILE:(t + 1) * W0_TILE])
                for cc in range(CH_PER_TILE):
                    c = t * CH_PER_TILE + cc
                    nc.tensor.matmul(
                        v0ps[:],
                        xn16[:, c:c + 1],
                        w0tile[:, cc * 128:(cc + 1) * 128],
                        start=(c == 0),
                        stop=(c == C - 1),
                    )

            v0sb = sp.tile([1, H], FP)
            nc.vector.tensor_copy(v0sb[:], v0ps[:])
            nc.sync.dma_start(outp_d[:], v0sb[:])

    nc.compile()
    return nc


def _build_b():
    """Launch B: natural fp16 slab, partial-reduce + chain + matvec2."""
    nc = bacc.Bacc("TRN2", target_bir_lowering=False, debug=False,
                   num_devices=N_CORES)

    w0_d = nc.dram_tensor("w0", [H, N_LOC], F16, kind="ExternalInput").ap()
    sm_d = nc.dram_tensor("smalls", [128, SB], FP, kind="ExternalInput").ap()

    outj_d = nc.dram_tensor("out_j", [128, C], FP, kind="ExternalOutput").ap()
    outv_d = nc.dram_tensor("out_v", [1, 1], FP, kind="ExternalOutput").ap()

    with tile.TileContext(nc) as tc:
        with tc.tile_pool(name="w0", bufs=N_W0_TILES) as w0p, \
             tc.tile_pool(name="small", bufs=1) as sp, \
             tc.tile_pool(name="ps", bufs=2, space="PSUM") as pp, \
             tc.tile_pool(name="psj", bufs=1, space="PSUM") as pjp, \
             tc.tile_pool(name="psw", bufs=1, space="PSUM") as pwp:

            _warmup_pe(nc, sp, pwp)

            # one packed small tensor, first in the sync queue
            sm = sp.tile([128, SB], FP)
            nc.sync.dma_start(sm[:], sm_d[:])
            w1t = sm[:, SB_W1T:SB_W1T + H]
            w2t = sm[:, SB_W2T:SB_W2T + H]
            w3t = sm[:, SB_W3T:SB_W3T + H]
            w1n = sm[:, SB_W1N:SB_W1N + H]
            w2n = sm[:, SB_W2N:SB_W2N + H]
            w3n = sm[:, SB_W3N:SB_W3N + H]
            b0col = sm[:, SB_BCOL + 0:SB_BCOL + 1]
            b1col = sm[:, SB_BCOL + 1:SB_BCOL + 2]
            b2col = sm[:, SB_BCOL + 2:SB_BCOL + 3]
            b3col = sm[:, SB_BCOL + 3:SB_BCOL + 4]
            woutc = sm[:, SB_BCOL + 4:SB_BCOL + 5]
            boutc = sm[0:1, SB_BCOL + 5:SB_BCOL + 6]
            invT = sm[:, SB_INVT:SB_INVT + C]

            # ---- forward chain (all vectors as [128, 1] columns) ----
            v0c = sp.tile([H, 1], FP)
            nc.vector.tensor_reduce(v0c[:], sm[:, SB_PARTS:SB_PARTS + N_CORES],
                                    mybir.AxisListType.X, AOT.add)

            v1c = sp.tile([H, 1], FP)
            nc.scalar.activation(v1c[:], v0c[:], ACT.Tanh, bias=b0col)
            d0c = sp.tile([H, 1], FP)
            nc.vector.tensor_mul(d0c[:], v1c[:], v1c[:])
            nc.vector.tensor_scalar(d0c[:], d0c[:], -1.0, 1.0, AOT.mult, AOT.add)

            a1ps = pp.tile([H, 1], FP, tag="chain_ps")
            nc.tensor.matmul(a1ps[:], w1t, v1c[:], start=True, stop=True)
            v2c = sp.tile([H, 1], FP)
            nc.scalar.activation(v2c[:], a1ps[:], ACT.Tanh, bias=b1col)
            d1c = sp.tile([H, 1], FP)
            nc.vector.tensor_mul(d1c[:], v2c[:], v2c[:])
            nc.vector.tensor_scalar(d1c[:], d1c[:], -1.0, 1.0, AOT.mult, AOT.add)

            a2ps = pp.tile([H, 1], FP, tag="chain_ps")
            nc.tensor.matmul(a2ps[:], w2t, v2c[:], start=True, stop=True)
            v3c = sp.tile([H, 1], FP)
            nc.scalar.activation(v3c[:], a2ps[:], ACT.Tanh, bias=b2col)
            d2c = sp.tile([H, 1], FP)
            nc.vector.tensor_mul(d2c[:], v3c[:], v3c[:])
            nc.vector.tensor_scalar(d2c[:], d2c[:], -1.0, 1.0, AOT.mult, AOT.add)

            a3ps = pp.tile([H, 1], FP, tag="chain_ps")
            nc.tensor.matmul(a3ps[:], w3t, v3c[:], start=True, stop=True)
            a3c = sp.tile([H, 1], FP)
            nc.scalar.activation(a3c[:], a3ps[:], ACT.Identity, bias=b3col)

            voutps = pp.tile([1, 1], FP, tag="chain_ps")
            nc.tensor.matmul(voutps[:], woutc, a3c[:], start=True, stop=True)
            voutsb = sp.tile([1, 1], FP)
            nc.scalar.activation(voutsb[:], voutps[:], ACT.Identity, bias=boutc)
            nc.scalar.dma_start(outv_d[:], voutsb[:])

            # ---- backward chain, direct column form:
            # r3 = W3.T @ Wout.T; q2 = r3*d2; r2 = W2.T @ q2; q1 = r2*d1;
            # r1 = W1.T @ q1; r0 = r1*d0  (lhsT = natural W gives W.T @ rhs)
            r3cp = pp.tile([H, 1], FP, tag="chain_ps")
            nc.tensor.matmul(r3cp[:], w3n, woutc, start=True, stop=True)
            q2c = sp.tile([H, 1], FP)
            nc.vector.tensor_mul(q2c[:], r3cp[:], d2c[:])

            r2cp = pp.tile([H, 1], FP, tag="chain_ps")
            nc.tensor.matmul(r2cp[:], w2n, q2c[:], start=True, stop=True)
            q1c = sp.tile([H, 1], FP)
            nc.vector.tensor_mul(q1c[:], r2cp[:], d1c[:])

            r1cp = pp.tile([H, 1], FP, tag="chain_ps")
            nc.tensor.matmul(r1cp[:], w1n, q1c[:], start=True, stop=True)
            r0c = sp.tile([H, 1], FP)
            nc.vector.tensor_mul(r0c[:], r1cp[:], d0c[:])
            r016 = sp.tile([H, 1], F16)
            nc.vector.tensor_copy(r016[:], r0c[:])

            # ---- matvec2: J[c*128+k] = sum_h W0[h, c*128+k] * r0[h] ----
            jt = pjp.tile([128, C], FP)
            jts = sp.tile([128, C], FP)
            for t in range(N_W0_TILES):
                w0tile = w0p.tile([128, W0_TILE], F16, tag="w0tile")
                eng = getattr(nc, _QUEUES[t % 3])
                eng.dma_start(w0tile[:], w0_d[:, t * W0_TILE:(t + 1) * W0_TILE])
                for cc in range(CH_PER_TILE):
                    c = t * CH_PER_TILE + cc
                    nc.tensor.matmul(
                        jt[:, c:c + 1],
                        w0tile[:, cc * 128:(cc + 1) * 128],
                        r016[:],
                        start=True,
                        stop=True,
                    )
                # store finished halves while the later tiles still stream
                if t == N_W0_TILES // 2 - 1:
                    h0 = C // 2
                    nc.vector.tensor_mul(jts[:, :h0], jt[:, :h0], invT[:, :h0])
                    nc.scalar.dma_start(outj_d[:, :h0], jts[:, :h0])
            h0 = C // 2
            nc.vector.tensor_mul(jts[:, h0:], jt[:, h0:], invT[:, h0:])
            nc.scalar.dma_start(outj_d[:, h0:], jts[:, h0:])

    nc.compile()
    return nc


def _get_kernels():
    if "nc_a" not in _CACHE:
        _CACHE["nc_a"] = _build_a()
        _CACHE["nc_b"] = _build_b()
    return _CACHE["nc_a"], _CACHE["nc_b"]


def kernel(**inputs):
    nc_a, nc_b = _get_kernels()
    f = np.float32

    state = np.asarray(inputs["state"], f).reshape(1, N_STATE)
    x_max = np.asarray(inputs["x_max"], f).reshape(N_STATE)
    x_min = np.asarray(inputs["x_min"], f).reshape(N_STATE)
    W0 = np.asarray(inputs["W0"], f)
    W1 = np.asarray(inputs["W1"], f)
    W2 = np.asarray(inputs["W2"], f)
    W3 = np.asarray(inputs["W3"], f)
    Wout = np.asarray(inputs["Wout"], f).reshape(1, H)
    b0 = np.asarray(inputs["b0"], f).reshape(H)
    b1 = np.asarray(inputs["b1"], f).reshape(H)
    b2 = np.asarray(inputs["b2"], f).reshape(H)
    b3 = np.asarray(inputs["b3"], f).reshape(H)
    bout = np.asarray(inputs["bout"], f).reshape(1)

    W016 = W0.astype(np.float16)
    x_center = (x_max + x_min) * 0.5
    x_range = (x_max - x_min) * 0.5
    inv_range = 1.0 / x_range
    x_norm = ((state[0] - x_center) * inv_range).astype(np.float16)

    in_maps_a = []
    for i in range(N_CORES):
        sl = slice(i * N_LOC, (i + 1) * N_LOC)
        w0t = np.ascontiguousarray(
            W016[:, sl].reshape(H, C, 128).transpose(2, 1, 0)
        ).reshape(128, C * H)
        in_maps_a.append({
            "w0t": w0t,
            "xn16": np.ascontiguousarray(x_norm[sl].reshape(C, 128).T),
        })

    trace = bool(int(os.environ.get("KERNEL_TRACE", "0")))
    res_a = bass_utils.run_bass_kernel_spmd(
        nc_a, in_maps_a, core_ids=list(range(N_CORES)), trace=trace
    )
    _CACHE["res_a"] = res_a

    # pure gather: stack the 8 per-core partial rows into [H, 8] columns
    parts = np.stack([res_a.results[i]["out_p"][0] for i in range(N_CORES)], axis=1)

    sm_base = np.zeros((128, SB), f)
    sm_base[:, SB_PARTS:SB_PARTS + N_CORES] = parts
    sm_base[:, SB_W1T:SB_W1T + H] = W1.T
    sm_base[:, SB_W2T:SB_W2T + H] = W2.T
    sm_base[:, SB_W3T:SB_W3T + H] = W3.T
    sm_base[:, SB_W1N:SB_W1N + H] = W1
    sm_base[:, SB_W2N:SB_W2N + H] = W2
    sm_base[:, SB_W3N:SB_W3N + H] = W3
    sm_base[:, SB_BCOL + 0] = b0
    sm_base[:, SB_BCOL + 1] = b1
    sm_base[:, SB_BCOL + 2] = b2
    sm_base[:, SB_BCOL + 3] = b3
    sm_base[:, SB_BCOL + 4] = Wout[0]
    sm_base[0, SB_BCOL + 5] = bout[0]

    in_maps_b = []
    for i in range(N_CORES):
        sl = slice(i * N_LOC, (i + 1) * N_LOC)
        sm = sm_base.copy()
        sm[:, SB_INVT:SB_INVT + C] = inv_range[sl].reshape(C, 128).T
        in_maps_b.append({
            "w0": np.ascontiguousarray(W016[:, sl]),
            "smalls": sm,
        })

    res_b = bass_utils.run_bass_kernel_spmd(
        nc_b, in_maps_b, core_ids=list(range(N_CORES)), trace=trace
    )
    _CACHE["res_b"] = res_b

    out = np.empty((1, N_STATE + 1), np.float32)
    out[0, 0] = float(np.asarray(res_b.results[0]["out_v"]).reshape(()))
    for i in range(N_CORES):
        jt = np.asarray(res_b.results[i]["out_j"])  # [k, c]
        out[0, 1 + i * N_LOC:1 + (i + 1) * N_LOC] = jt.T.reshape(-1)
    return out
